# revision 4
# baseline (speedup 1.0000x reference)
"""Bass/Trainium2 kernel for nn_DiagonalTransfer.

Math: out[i, k] = logsumexp_j(D[i, j] + xx[j, k]) with D = diag(diag)
(zeros off-diagonal).  Since D is diagonal plus a zero background:

    out[i, k] = log( sum_j exp(xx[j, k]) + exp(xx[i, k]) * (exp(diag[i]) - 1) )
              = log( S[k] + E[i, k] * c[i] )

with S[k] = sum_j exp(xx[j, k]), E = exp(xx), c = expm1(diag).
All terms rewritten this way stay positive: S - E[i,k] >= sum_{j != i} E[j,k] > 0.

Device strategy (8 cores, data parallel over the K observation dim):
  - Host computes c = expm1(diag) and transposes xx -> xxT (K, N) so each
    core receives a contiguous (K/8, N) shard with k on partitions.
  - Per [128, N] tile: ScalarE Exp with accum_out yields E and the
    per-partition row sums S[k] in one pass; VectorE multiplies by the
    broadcast c row; ScalarE Ln with bias=S fuses the add and the log.
  - Output is the transposed shard; host re-transposes and concatenates.
"""

import numpy as np

import concourse.bass as bass
import concourse.bacc as bacc
import concourse.tile as tile
from concourse import mybir
from concourse.bass_utils import run_bass_kernel_spmd

N = 1024          # num_states (rows of xx, length of diag)
K = 8192          # observation columns of xx
NCORES = 8
KS = K // NCORES  # columns per core
P = 128           # SBUF partitions
NT = KS // P      # k-tiles per core

_cached_nc = None


def build_bass():
    """Per-core program: xxT shard (KS, N) + c (N,) -> outT shard (KS, N)."""
    nc = bacc.Bacc("TRN2", target_bir_lowering=False, debug=False)
    xxT = nc.declare_dram_parameter("xxT", [KS, N], mybir.dt.float32, isOutput=False)
    cvec = nc.declare_dram_parameter("c", [N], mybir.dt.float32, isOutput=False)
    outT = nc.declare_dram_parameter("outT", [KS, N], mybir.dt.float32, isOutput=True)

    with tile.TileContext(nc) as tc:
        with (
            tc.tile_pool(name="const", bufs=1) as const_pool,
            tc.tile_pool(name="work", bufs=3) as work,
            tc.tile_pool(name="sums", bufs=4) as sums,
            tc.tile_pool(name="outs", bufs=3) as outs,
        ):
            # c broadcast to all 128 partitions, once.
            c_b = const_pool.tile([P, N], mybir.dt.float32)
            nc.sync.dma_start(out=c_b[:], in_=cvec[:].partition_broadcast(P))

            for t in range(NT):
                x_t = work.tile([P, N], mybir.dt.float32, tag="x")
                nc.sync.dma_start(out=x_t[:], in_=xxT[t * P : (t + 1) * P, :])

                e_t = work.tile([P, N], mybir.dt.float32, tag="e")
                s_t = sums.tile([P, 1], mybir.dt.float32, tag="s")
                # E = exp(x); accum_out gives S[k] = sum_i E[k, i] per partition.
                nc.scalar.activation(
                    out=e_t[:],
                    in_=x_t[:],
                    func=mybir.ActivationFunctionType.Exp,
                    accum_out=s_t[:],
                )
                # E *= c (broadcast along partitions)
                nc.vector.tensor_mul(out=e_t[:], in0=e_t[:], in1=c_b[:])
                # out = ln(E*c + S)
                o_t = outs.tile([P, N], mybir.dt.float32, tag="o")
                nc.scalar.activation(
                    out=o_t[:],
                    in_=e_t[:],
                    func=mybir.ActivationFunctionType.Ln,
                    bias=s_t[:, 0:1],
                    scale=1.0,
                )
                nc.sync.dma_start(out=outT[t * P : (t + 1) * P, :], in_=o_t[:])
    nc.compile()
    return nc


def _get_nc():
    global _cached_nc
    if _cached_nc is None:
        _cached_nc = build_bass()
    return _cached_nc


def run(diag, xx, **spmd_kwargs):
    """Run on 8 cores; returns (out, BassKernelResults)."""
    diag = np.asarray(diag, dtype=np.float32)
    xx = np.asarray(xx, dtype=np.float32)
    c = np.expm1(diag.astype(np.float64)).astype(np.float32)
    xxT = np.ascontiguousarray(xx.T)  # (K, N)
    in_maps = [
        {"xxT": np.ascontiguousarray(xxT[i * KS : (i + 1) * KS]), "c": c}
        for i in range(NCORES)
    ]
    res = run_bass_kernel_spmd(_get_nc(), in_maps, list(range(NCORES)), **spmd_kwargs)
    outT = np.concatenate([res.results[i]["outT"] for i in range(NCORES)], axis=0)
    out = np.ascontiguousarray(outT.T).astype(np.float32)
    return out, res


def kernel(diag, xx):
    out, _ = run(diag, xx)
    return out


# revision 5
# speedup vs baseline: 1.0091x; 1.0091x over previous
"""Bass/Trainium2 kernel for nn_DiagonalTransfer.

Math: out[i, k] = logsumexp_j(D[i, j] + xx[j, k]) with D = diag(diag)
(zeros off-diagonal).  Since D is diagonal plus a zero background:

    out[i, k] = log( sum_j exp(xx[j, k]) + exp(xx[i, k]) * (exp(diag[i]) - 1) )
              = log( S[k] + E[i, k] * c[i] )

with S[k] = sum_j exp(xx[j, k]), E = exp(xx), c = expm1(diag).
All terms rewritten this way stay positive: S - E[i,k] >= sum_{j != i} E[j,k] > 0.

Device strategy (8 cores, data parallel over the K observation dim):
  - Host computes c = expm1(diag) and transposes xx -> xxT (K, N) so each
    core receives a contiguous (K/8, N) shard with k on partitions.
  - Per [128, N] tile: ScalarE Exp with accum_out yields E and the
    per-partition row sums S[k] in one pass; VectorE multiplies by the
    broadcast c row; ScalarE Ln with bias=S fuses the add and the log.
  - Output is the transposed shard; host re-transposes and concatenates.
"""

import numpy as np

import concourse.bass as bass
import concourse.bacc as bacc
import concourse.tile as tile
from concourse import mybir
from concourse.bass_utils import run_bass_kernel_spmd

N = 1024          # num_states (rows of xx, length of diag)
K = 8192          # observation columns of xx
NCORES = 8
KS = K // NCORES  # columns per core
P = 128           # SBUF partitions
NT = KS // P      # k-tiles per core

_cached_nc = None


def build_bass():
    """Per-core program: xxT shard (KS, N) + c (N,) -> outT shard (KS, N)."""
    nc = bacc.Bacc("TRN2", target_bir_lowering=False, debug=False)
    xxT = nc.declare_dram_parameter("xxT", [KS, N], mybir.dt.float32, isOutput=False)
    cvec = nc.declare_dram_parameter("c", [N], mybir.dt.float32, isOutput=False)
    outT = nc.declare_dram_parameter("outT", [KS, N], mybir.dt.float32, isOutput=True)

    with tile.TileContext(nc) as tc:
        with (
            tc.tile_pool(name="const", bufs=1) as const_pool,
            tc.tile_pool(name="work", bufs=3) as work,
            tc.tile_pool(name="sums", bufs=4) as sums,
            tc.tile_pool(name="outs", bufs=3) as outs,
        ):
            # Preload the combined exp+ln activation table set so the
            # alternating Exp/Ln stream needs no per-tile table reloads.
            # act_func_set_id 6 == "natural_log_exp_and_others" for gen3.
            with tc.high_priority():
                nc.scalar.add_instruction(
                    mybir.InstLoadActFuncSet(
                        name=nc.get_next_instruction_name(),
                        ins=[],
                        outs=[],
                        act_func_set_id=6,
                    )
                )

            # c broadcast to all 128 partitions, once.
            c_b = const_pool.tile([P, N], mybir.dt.float32)
            nc.sync.dma_start(out=c_b[:], in_=cvec[:].partition_broadcast(P))

            for t in range(NT):
                x_t = work.tile([P, N], mybir.dt.float32, tag="x")
                nc.sync.dma_start(out=x_t[:], in_=xxT[t * P : (t + 1) * P, :])

                e_t = work.tile([P, N], mybir.dt.float32, tag="e")
                s_t = sums.tile([P, 1], mybir.dt.float32, tag="s")
                # E = exp(x); accum_out gives S[k] = sum_i E[k, i] per partition.
                nc.scalar.activation(
                    out=e_t[:],
                    in_=x_t[:],
                    func=mybir.ActivationFunctionType.Exp,
                    accum_out=s_t[:],
                )
                # E *= c (broadcast along partitions)
                nc.vector.tensor_mul(out=e_t[:], in0=e_t[:], in1=c_b[:])
                # out = ln(E*c + S)
                o_t = outs.tile([P, N], mybir.dt.float32, tag="o")
                nc.scalar.activation(
                    out=o_t[:],
                    in_=e_t[:],
                    func=mybir.ActivationFunctionType.Ln,
                    bias=s_t[:, 0:1],
                    scale=1.0,
                )
                nc.sync.dma_start(out=outT[t * P : (t + 1) * P, :], in_=o_t[:])
    nc.compile()
    return nc


def _get_nc():
    global _cached_nc
    if _cached_nc is None:
        _cached_nc = build_bass()
    return _cached_nc


def run(diag, xx, **spmd_kwargs):
    """Run on 8 cores; returns (out, BassKernelResults)."""
    diag = np.asarray(diag, dtype=np.float32)
    xx = np.asarray(xx, dtype=np.float32)
    c = np.expm1(diag.astype(np.float64)).astype(np.float32)
    xxT = np.ascontiguousarray(xx.T)  # (K, N)
    in_maps = [
        {"xxT": np.ascontiguousarray(xxT[i * KS : (i + 1) * KS]), "c": c}
        for i in range(NCORES)
    ]
    res = run_bass_kernel_spmd(_get_nc(), in_maps, list(range(NCORES)), **spmd_kwargs)
    outT = np.concatenate([res.results[i]["outT"] for i in range(NCORES)], axis=0)
    out = np.ascontiguousarray(outT.T).astype(np.float32)
    return out, res


def kernel(diag, xx):
    out, _ = run(diag, xx)
    return out


# revision 7
# speedup vs baseline: 1.2142x; 1.2032x over previous
"""Bass/Trainium2 kernel for nn_DiagonalTransfer.

Math: out[i, k] = logsumexp_j(D[i, j] + xx[j, k]) with D = diag(diag)
(zeros off-diagonal).  Since D is diagonal plus a zero background:

    out[i, k] = log( sum_j exp(xx[j, k]) + exp(xx[i, k]) * (exp(diag[i]) - 1) )
              = log( S[k] + E[i, k] * c[i] )

with S[k] = sum_j exp(xx[j, k]), E = exp(xx), c = expm1(diag).
All terms rewritten this way stay positive: S - E[i,k] >= sum_{j != i} E[j,k] > 0.

Device strategy (8 cores, data parallel over the K observation dim):
  - Host computes c = expm1(diag) and transposes xx -> xxT (K, N) so each
    core receives a contiguous (K/8, N) shard with k on partitions.
  - Per [128, N] tile: ScalarE Exp with accum_out yields E and the
    per-partition row sums S[k] in one pass; VectorE multiplies by the
    broadcast c row; ScalarE Ln with bias=S fuses the add and the log.
  - Output is the transposed shard; host re-transposes and concatenates.
"""

import numpy as np

import concourse.bass as bass
import concourse.bacc as bacc
import concourse.tile as tile
from concourse import mybir
from concourse.bass_utils import run_bass_kernel_spmd

N = 1024          # num_states (rows of xx, length of diag)
K = 8192          # observation columns of xx
NCORES = 8
KS = K // NCORES  # columns per core
P = 128           # SBUF partitions
NT = KS // P      # k-tiles per core

_cached_nc = None


def build_bass():
    """Per-core program: xxT shard (KS, N) + c (N,) -> outT shard (KS, N)."""
    nc = bacc.Bacc("TRN2", target_bir_lowering=False, debug=False)
    xxT = nc.declare_dram_parameter("xxT", [KS, N], mybir.dt.float32, isOutput=False)
    cvec = nc.declare_dram_parameter("c", [N], mybir.dt.float32, isOutput=False)
    outT = nc.declare_dram_parameter("outT", [KS, N], mybir.dt.float32, isOutput=True)

    # Two k-tiles ride in one 1 MiB DMA: SBUF [128, 2, N] where chunk j of
    # partition p holds DRAM row (t*2+j)*128 + p.
    B = 2                 # k-tiles per DMA batch
    NB = NT // B          # batches per core

    with tile.TileContext(nc) as tc:
        with (
            tc.tile_pool(name="const", bufs=1) as const_pool,
            tc.tile_pool(name="work", bufs=4) as work,
            tc.tile_pool(name="sums", bufs=8) as sums,
            tc.tile_pool(name="outs", bufs=4) as outs,
        ):
            # Preload the combined exp+ln activation table set so the
            # alternating Exp/Ln stream needs no per-tile table reloads.
            # act_func_set_id 6 == "natural_log_exp_and_others" for gen3.
            with tc.high_priority():
                nc.scalar.add_instruction(
                    mybir.InstLoadActFuncSet(
                        name=nc.get_next_instruction_name(),
                        ins=[],
                        outs=[],
                        act_func_set_id=6,
                    )
                )

            # c twice along the free dim, broadcast to all partitions (issued
            # on the ACT HWDGE ring to keep the SP ring free for loads).
            c_b = const_pool.tile([P, B, N], mybir.dt.float32)
            c_ap = cvec[:]
            c_src = bass.AP(
                tensor=c_ap.tensor, offset=c_ap.offset, ap=[[0, P], [0, B], [1, N]]
            )
            nc.scalar.dma_start(out=c_b[:], in_=c_src)

            xxT_b = xxT.rearrange("(nb b p) n -> nb p b n", b=B, p=P)
            outT_b = outT.rearrange("(nb b p) n -> nb p b n", b=B, p=P)

            for t in range(NB):
                x_t = work.tile([P, B, N], mybir.dt.float32, tag="x")
                nc.sync.dma_start(out=x_t[:], in_=xxT_b[t])

                e_t = work.tile([P, B, N], mybir.dt.float32, tag="e")
                s_t = sums.tile([P, B], mybir.dt.float32, tag="s")
                # E = exp(x); accum_out gives S[k] = sum_i E[k, i] per
                # partition. One activation per chunk: the accumulator must
                # not mix the two k-rows sharing a partition.
                for j in range(B):
                    nc.scalar.activation(
                        out=e_t[:, j, :],
                        in_=x_t[:, j, :],
                        func=mybir.ActivationFunctionType.Exp,
                        accum_out=s_t[:, j : j + 1],
                    )
                # E *= c (broadcast along partitions), both chunks at once
                nc.vector.tensor_mul(out=e_t[:], in0=e_t[:], in1=c_b[:])
                # out = ln(E*c + S)
                o_t = outs.tile([P, B, N], mybir.dt.float32, tag="o")
                for j in range(B):
                    nc.scalar.activation(
                        out=o_t[:, j, :],
                        in_=e_t[:, j, :],
                        func=mybir.ActivationFunctionType.Ln,
                        bias=s_t[:, j : j + 1],
                        scale=1.0,
                    )
                nc.gpsimd.dma_start(out=outT_b[t], in_=o_t[:])
    nc.compile()
    return nc


def _get_nc():
    global _cached_nc
    if _cached_nc is None:
        _cached_nc = build_bass()
    return _cached_nc


def run(diag, xx, **spmd_kwargs):
    """Run on 8 cores; returns (out, BassKernelResults)."""
    diag = np.asarray(diag, dtype=np.float32)
    xx = np.asarray(xx, dtype=np.float32)
    c = np.expm1(diag.astype(np.float64)).astype(np.float32)
    xxT = np.ascontiguousarray(xx.T)  # (K, N)
    in_maps = [
        {"xxT": np.ascontiguousarray(xxT[i * KS : (i + 1) * KS]), "c": c}
        for i in range(NCORES)
    ]
    res = run_bass_kernel_spmd(_get_nc(), in_maps, list(range(NCORES)), **spmd_kwargs)
    outT = np.concatenate([res.results[i]["outT"] for i in range(NCORES)], axis=0)
    out = np.ascontiguousarray(outT.T).astype(np.float32)
    return out, res


def kernel(diag, xx):
    out, _ = run(diag, xx)
    return out
